# revision 19
# baseline (speedup 1.0000x reference)
"""Trainium2 Bass kernel for a dense transformer block (B=4, T=1024, C=1024, H=16).

Sharding: 2 cores per batch element (8 cores / 4 batches). Each core computes
K/V (+LN1) for its full batch but only 4 of the 8 query blocks of 128 rows.
Query blocks are interleaved ({7,4,3,0} on even cores, {6,5,2,1} on odd) so the
causal-attention work is balanced; the compiled program is identical on every
core (SPMD) - per-core behaviour comes only from input data (x slice, gathered
query rows, causal-mask tiles).

v3 layout/scheduling:
- LN gamma/beta folded into the next matmul's weights+bias on the HOST, so the
  device LN is only (x-m)*rstd.
- Q/K/V/O projections run in fp8e4m3 with DoubleRow (weights host-scaled x16,
  un-scaled in the bias-apply). Scores/AV and the MLP stay bf16.
- h transposes done by the DMA XBAR (dma_start_transpose) instead of the PE
  (XBAR is 16-bit, so transpose to a bf16 staging tile, then GpSimd casts to
  the fp8 hT).
- V projection is streamed per row-block right after that block's
  LN+transpose, K projection in two halves, so the tensor engine is densely
  busy early (keeps the HAM clock-gate at full rate).
- Two DMA queues: sync carries the x stream + small data, Activation carries
  all weight slabs, ordered so nothing compute-critical queues behind bulk
  weights. w1/w2 are prefetched during attention / MLP1.
- Attention scores for all 8 heads of a group land in one PSUM tile
  [128, 8, 128] (hh-major so the concurrent row-tiled pair writes distinct
  PSUM banks), exp'd by ONE 1024-wide activation per (slot, group, kb).
- Causal masks applied by a single broadcast scalar_tensor_tensor on the DVE.
- Softmax denominators: den row -> tiny PE transposes -> one batched [128,8]
  reciprocal -> PE transpose back -> K=1 matmul broadcast -> one fused
  normalize multiply per (slot, group) into the head-interleaved yT_all
  (wo rows host-permuted to match). Norm scratch shares the score pool ring.
"""
import os
import sys

for _p in ("/opt/trn_rl_repo", "/root/.axon_site/_ro/trn_rl_repo"):
    if os.path.isdir(_p) and _p not in sys.path:
        sys.path.insert(0, _p)

from contextlib import ExitStack

import ml_dtypes
import numpy as np

import concourse.bass as bass
import concourse.tile as tile
from concourse import mybir
from concourse.bass_utils import run_bass_kernel_spmd
from concourse.masks import make_identity

F32 = mybir.dt.float32
BF16 = mybir.dt.bfloat16
FP8 = mybir.dt.float8e4
PM_DR = mybir.MatmulPerfMode.DoubleRow
WSCALE = 16.0
WINV = 1.0 / WSCALE
AF = mybir.ActivationFunctionType
OP = mybir.AluOpType

B, T, C, H, D = 4, 1024, 1024, 16, 64
F = 4 * C                       # MLP hidden
NB = T // 128                   # 8 row blocks per batch
NSLOT = 4                       # query blocks per core
KMAX = [8, 6, 4, 2]             # k-blocks computed per slot (max over both cores)
QBLOCKS = [[7, 4, 3, 0], [6, 5, 2, 1]]  # global q-block per slot, by core parity
# (slot, kb) pairs that need a data mask (kb below min over parities: always allow)
MASKED = [(0, 6), (0, 7), (1, 4), (1, 5), (2, 2), (2, 3), (3, 0), (3, 1)]
EPS = 1e-5
NEG = -1e9


def build_nc():
    nc = bass.Bass("TRN2")

    # ---- DRAM I/O ----------------------------------------------------------
    xb = nc.dram_tensor("xb", [T, C], F32, kind="ExternalInput")     # full batch rows
    xq = nc.dram_tensor("xq", [512, C], F32, kind="ExternalInput")   # gathered q rows
    masks = nc.dram_tensor("masks", [8, 128, 128], F32, kind="ExternalInput")
    wq = nc.dram_tensor("wq", [C, C], FP8, kind="ExternalInput")
    wk = nc.dram_tensor("wk", [C, C], FP8, kind="ExternalInput")
    wv = nc.dram_tensor("wv", [C, C], FP8, kind="ExternalInput")
    wo = nc.dram_tensor("wo", [C, C], FP8, kind="ExternalInput")
    w1 = nc.dram_tensor("w1", [C, F], BF16, kind="ExternalInput")
    w2 = nc.dram_tensor("w2", [F, C], BF16, kind="ExternalInput")
    bq = nc.dram_tensor("bq", [C], F32, kind="ExternalInput")
    bk = nc.dram_tensor("bk", [C], F32, kind="ExternalInput")
    bv = nc.dram_tensor("bv", [C], F32, kind="ExternalInput")
    bo = nc.dram_tensor("bo", [C], F32, kind="ExternalInput")
    b1 = nc.dram_tensor("b1", [F], F32, kind="ExternalInput")
    b2 = nc.dram_tensor("b2", [C], F32, kind="ExternalInput")
    out = nc.dram_tensor("out", [512, C], F32, kind="ExternalOutput")

    with tile.TileContext(nc) as tc, ExitStack() as ctx:
        consts = ctx.enter_context(tc.tile_pool(name="consts", bufs=1))
        small = ctx.enter_context(tc.tile_pool(name="small", bufs=2))

        # ---- constants (no DMA, or tiny/early-needed loads) ---------------
        ones_row = consts.tile([1, 64], BF16, tag="ones_row", name="ones_row")
        nc.vector.memset(ones_row, 1.0)
        eps_col = consts.tile([128, 1], F32, tag="eps", name="eps")
        nc.vector.memset(eps_col, EPS)
        ident_f32 = consts.tile([128, 128], F32, tag="ident", name="ident")
        make_identity(nc, ident_f32)
        one_f32 = consts.tile([1, 1], F32, tag="one1", name="one1")
        nc.vector.memset(one_f32, 1.0)

        def load_cols(dram, nblk, tag):
            t = consts.tile([128, nblk], F32, tag=tag)
            nc.sync.dma_start(out=t, in_=dram.rearrange("(a p) -> p a", p=128))
            return t

        def load_bcast(dram, tag):
            t = consts.tile([128, C], F32, tag=tag)
            nc.sync.dma_start(
                out=t,
                in_=dram.rearrange("(one c) -> one c", one=1).partition_broadcast(128))
            return t

        xmid = [consts.tile([128, C], F32, tag=f"xmid{j}", name=f"xmid{j}") for j in range(4)]

        pw1 = ctx.enter_context(tc.tile_pool(name="p_w1", bufs=2))
        w1c = {}

        def load_w1(chunk):
            t = pw1.tile([128, 8, C], BF16, tag="w1c", name="w1c")
            nc.scalar.dma_start(out=t, in_=w1[:, chunk * C:(chunk + 1) * C]
                                .rearrange("(a p) c -> p a c", p=128))
            w1c[chunk] = t

        att_ctx = ExitStack()
        p_att = att_ctx.enter_context(tc.tile_pool(name="p_att", bufs=1))
        p_w = att_ctx.enter_context(tc.tile_pool(name="p_w", bufs=2))

        # attention operands (filled in phase 1)
        qT = p_att.tile([128, 8, 512], BF16, tag="qT", name="qT")
        kT = p_att.tile([128, 8, T], BF16, tag="kT", name="kT")
        vaug = [p_att.tile([128, 16, 65], BF16, tag=f"vaug{t}", name=f"vaug{t}") for t in range(8)]
        # yT_all[hg*64+d, h8, q] = normalized y for head hg*8+h8, dim d.
        # (wo rows are host-permuted to match this head-interleaved layout.)
        yT_all = p_att.tile([128, 8, 512], FP8, tag="yT", name="yT")

        def load_wslab(dram, name):
            t = p_w.tile([128, 8, C], FP8, tag="wslab", name=name)
            nc.scalar.dma_start(out=t, in_=dram.rearrange("(a p) c -> p a c", p=128))
            return t

        # first-needed loads: V weights (ACT queue) + V bias broadcast (sync)
        wv_sb = load_wslab(wv, "wv_sb")
        BV = load_bcast(bv, "BV")

        # ==== phase 1: streamed LN1 -> hT (DMA transpose) -> V/K/Q proj =====
        h1_ctx = ExitStack()
        p_h1 = h1_ctx.enter_context(tc.tile_pool(name="p_h1", bufs=1))
        hT = p_h1.tile([128, 8, 1536], FP8, tag="hT", name="hT")

        ph1s = h1_ctx.enter_context(tc.tile_pool(name="p_h1s", bufs=2))
        ps_qkv = h1_ctx.enter_context(tc.tile_pool(name="ps_qkv", bufs=4, space="PSUM"))

        def ln_block(x_ap, col_off):
            """LN (no gamma/beta) of [128, C] rows -> hT[:, :, col_off:+128]."""
            stats = ph1s.tile([128, 2, 6], F32, tag="ln_stats", name="ln_stats")
            for s in range(2):
                nc.vector.bn_stats(out=stats[:, s, :], in_=x_ap[:, s * 512:(s + 1) * 512])
            mv = ph1s.tile([128, 2], F32, tag="ln_mv", name="ln_mv")
            nc.vector.bn_aggr(out=mv, in_=stats)
            std = ph1s.tile([128, 1], F32, tag="ln_std", name="ln_std")
            nc.scalar.activation(out=std, in_=mv[:, 1:2], func=AF.Sqrt, bias=eps_col)
            rstd = ph1s.tile([128, 1], F32, tag="ln_rstd", name="ln_rstd")
            nc.vector.reciprocal(out=rstd, in_=std)
            h_rows = ph1s.tile([128, C], BF16, tag="h_rows", name="h_rows")
            nc.vector.tensor_scalar(out=h_rows, in0=x_ap, scalar1=mv[:, 0:1],
                                    scalar2=rstd, op0=OP.subtract, op1=OP.mult)
            stage_bf = ph1s.tile([128, 8, 128], BF16, tag="tstage", name="tstage")
            nc.scalar.dma_start_transpose(out=stage_bf, in_=h_rows)
            nc.gpsimd.tensor_copy(out=hT[:, :, col_off:col_off + 128], in_=stage_bf)

        def k_proj(nt):
            for co in range(8):
                ps = ps_qkv.tile([128, 512], F32, tag="mm", name="mm")
                for ci in range(0, 8, 2):
                    nc.tensor.matmul(
                        ps, lhsT=wk_sb[:, ci:ci + 2, co * 128:(co + 1) * 128],
                        rhs=hT[:, ci:ci + 2, nt * 512:(nt + 1) * 512],
                        start=(ci == 0), stop=(ci == 6), perf_mode=PM_DR)
                nc.vector.tensor_scalar(
                    out=kT[:, co, nt * 512:(nt + 1) * 512], in0=ps,
                    scalar1=WINV, scalar2=bkc[:, co:co + 1],
                    op0=OP.mult, op1=OP.add)

        for r in range(NB):
            x_t = ph1s.tile([128, C], F32, tag="x_t", name="x_t")
            nc.sync.dma_start(out=x_t, in_=xb[r * 128:(r + 1) * 128, :])
            ln_block(x_t, r * 128)
            # V projection for this row block (+bias), interleaved + ones col
            nc.vector.memset(vaug[r][:, :, 64:65], 1.0)
            for nt in range(2):
                ps = ps_qkv.tile([128, 512], F32, tag="mm", name="mm")
                for ci in range(0, 8, 2):
                    nc.tensor.matmul(
                        ps, lhsT=hT[:, ci:ci + 2, r * 128:(r + 1) * 128],
                        rhs=wv_sb[:, ci:ci + 2, nt * 512:(nt + 1) * 512],
                        start=(ci == 0), stop=(ci == 6), perf_mode=PM_DR)
                nc.vector.scalar_tensor_tensor(
                    out=vaug[r][:, nt * 8:(nt + 1) * 8, 0:64],
                    in0=ps.rearrange("p (h d) -> p h d", d=64),
                    scalar=WINV,
                    in1=BV[:, nt * 512:(nt + 1) * 512]
                        .rearrange("p (h d) -> p h d", d=64),
                    op0=OP.mult, op1=OP.add)
            # staggered loads on both queues, ordered by first use
            if r == 0:
                wk_sb = load_wslab(wk, "wk_sb")
                bkc = load_cols(bk, 8, "bkc")
            if r == 2:
                bqc = load_cols(bq, 8, "bqc")
            if r == 3:
                k_proj(0)
                wq_sb = load_wslab(wq, "wq_sb")
            if r == 4:
                xq_sb = p_att.tile([128, 4, C], F32, tag="xq", name="xq")
                nc.sync.dma_start(out=xq_sb,
                                  in_=xq.rearrange("(j p) c -> p j c", p=128))
            if r == 5:
                mask_sb = p_att.tile([128, 8, 128], F32, tag="masks", name="masks")
                nc.sync.dma_start(out=mask_sb, in_=masks.rearrange("i p q -> p i q"))
            if r == 6:
                BO = load_bcast(bo, "BO")
            if r == 7:
                k_proj(1)
                wo_sb = load_wslab(wo, "wo_sb")

        for j in range(NSLOT):
            ln_block(xq_sb[:, j, :], 1024 + j * 128)

        # Q^T -> [C, 512]
        for co in range(8):
            ps = ps_qkv.tile([128, 512], F32, tag="mm", name="mm")
            for ci in range(0, 8, 2):
                nc.tensor.matmul(ps, lhsT=wq_sb[:, ci:ci + 2, co * 128:(co + 1) * 128],
                                 rhs=hT[:, ci:ci + 2, 1024:1536],
                                 start=(ci == 0), stop=(ci == 6), perf_mode=PM_DR)
            nc.vector.tensor_scalar(out=qT[:, co, :], in0=ps,
                                    scalar1=WINV, scalar2=bqc[:, co:co + 1],
                                    op0=OP.mult, op1=OP.add)

        h1_ctx.close()

        # prefetch the first two w1 chunks during attention (ACT queue)
        load_w1(0)
        load_w1(1)

        # ==== phase 2: attention ===========================================
        mask_idx = {sk: i for i, sk in enumerate(MASKED)}
        with tc.tile_pool(name="p_exp", bufs=8) as pexp, \
             tc.tile_pool(name="p_sm", bufs=2) as psm, \
             tc.tile_pool(name="ps_s", bufs=2, space="PSUM") as ps_s, \
             tc.tile_pool(name="ps_y", bufs=2, space="PSUM") as ps_y:
            for j in range(NSLOT):
              for hg in range(2):             # head groups of 8
                # scores for all 8 heads of the group: [128, hh*4+hp4, 128]
                # (hh-major so the concurrent row-tiled pair writes distinct
                # PSUM banks)
                expS = [pexp.tile([128, 8, 128], BF16, tag="expS", name="expS")
                        for _ in range(KMAX[j])]
                for kb in range(KMAX[j]):
                    s_ps = ps_s.tile([128, 8, 128], F32, tag="s_ps", name="s_ps")
                    for hp4 in range(4):
                        hp = 4 * hg + hp4
                        for hh in range(2):
                            nc.tensor.matmul(
                                s_ps[:, hh * 4 + hp4, :],
                                lhsT=kT[hh * 64:(hh + 1) * 64, hp,
                                        kb * 128:(kb + 1) * 128],
                                rhs=qT[hh * 64:(hh + 1) * 64, hp,
                                       j * 128:(j + 1) * 128],
                                start=True, stop=True,
                                tile_position=(64 * hh, 0))
                    if (j, kb) in mask_idx:
                        mi = mask_idx[(j, kb)]
                        m_b = mask_sb[:, mi, :].unsqueeze(1) \
                            .broadcast_to([128, 8, 128])
                        sm = psm.tile([128, 8, 128], F32, tag="sm", name="sm")
                        nc.vector.scalar_tensor_tensor(
                            out=sm, in0=s_ps, scalar=0.125, in1=m_b,
                            op0=OP.mult, op1=OP.add)
                        nc.scalar.activation(out=expS[kb], in_=sm, func=AF.Exp)
                    else:
                        nc.scalar.activation(out=expS[kb], in_=s_ps,
                                             func=AF.Exp, scale=0.125)
                # AV for all 8 heads -> one PSUM tile [65, h8, 128]
                # (row 64 = softmax denominator from the vaug ones column)
                yaug8 = ps_y.tile([65, 8, 128], F32, tag="yaug8", name="yaug8")
                for h8 in range(8):
                    h = 8 * hg + h8
                    for kb in range(KMAX[j]):
                        nc.tensor.matmul(
                            yaug8[:, h8, :], lhsT=vaug[kb][:, h, :],
                            rhs=expS[kb][:, (h8 % 2) * 4 + h8 // 2, :],
                            start=(kb == 0), stop=(kb == KMAX[j] - 1))
                # normalization: den row -> transpose to q-partitions ->
                # batched reciprocal -> transpose back -> broadcast -> 1 mul.
                # scratch shares the score-pool ring (no extra PSUM banks).
                den_row = small.tile([1, 8, 128], F32, tag="den_row",
                                     name="den_row")
                nc.vector.tensor_copy(out=den_row, in_=yaug8[64:65, :, :])
                scr = ps_s.tile([128, 8, 128], F32, tag="s_ps", name="scr")
                dcol = scr[:, 0, 0:8]
                for h8 in range(8):
                    nc.tensor.transpose(dcol[:, h8:h8 + 1],
                                        den_row[0:1, h8, :], one_f32)
                rq = small.tile([128, 8], F32, tag="rq", name="rq")
                nc.vector.reciprocal(out=rq, in_=dcol)
                rT_ps = scr[0:8, 2, :]
                nc.tensor.transpose(rT_ps, rq, ident_f32)
                rT_sb = small.tile([8, 128], BF16, tag="rT_sb", name="rT_sb")
                nc.vector.tensor_copy(out=rT_sb, in_=rT_ps)
                rrow = small.tile([1, 8, 128], BF16, tag="rrow", name="rrow")
                nc.sync.dma_start(out=rrow[0:1, :, :], in_=rT_sb[:, :])
                for half in range(2):
                    nc.tensor.matmul(
                        scr[0:64, 4 * half:4 * half + 4, :],
                        lhsT=ones_row,
                        rhs=rrow[0:1, 4 * half:4 * half + 4, :],
                        start=True, stop=True)
                rb8_sb = small.tile([64, 8, 128], BF16, tag="rb8_sb",
                                    name="rb8_sb")
                nc.scalar.mul(rb8_sb, scr[0:64, :, :], 1.0)
                nc.vector.tensor_mul(
                    out=yT_all[hg * 64:(hg + 1) * 64, :,
                               j * 128:(j + 1) * 128],
                    in0=yaug8[0:64, :, :], in1=rb8_sb)

        # ==== phase 3: output projection + residual ========================
        with tc.tile_pool(name="ps_pr", bufs=4, space="PSUM") as ps_pr:
            for j in range(NSLOT):
                for nt in range(2):
                    ps = ps_pr.tile([128, 512], F32, tag="prj", name="prj")
                    for ci in range(0, 8, 2):
                        nc.tensor.matmul(
                            ps, lhsT=yT_all[:, ci:ci + 2, j * 128:(j + 1) * 128],
                            rhs=wo_sb[:, ci:ci + 2, nt * 512:(nt + 1) * 512],
                            start=(ci == 0), stop=(ci == 6), perf_mode=PM_DR)
                    t1 = small.tile([128, 512], F32, tag="prt", name="prt")
                    nc.vector.scalar_tensor_tensor(
                        out=t1, in0=ps, scalar=WINV,
                        in1=BO[:, nt * 512:(nt + 1) * 512],
                        op0=OP.mult, op1=OP.add)
                    nc.vector.tensor_add(
                        xmid[j][:, nt * 512:(nt + 1) * 512], t1,
                        xq_sb[:, j, nt * 512:(nt + 1) * 512])

        att_ctx.close()

        # ==== phase 4: LN2 (DMA transpose) + MLP ===========================
        p_mlp = ctx.enter_context(tc.tile_pool(name="p_mlp", bufs=1))
        h2T = p_mlp.tile([128, 8, 512], BF16, tag="h2T", name="h2T")
        mT = p_mlp.tile([128, 32, 512], BF16, tag="mT", name="mT")

        # w2 quarters: 4 fresh buffers dispatched up front on the ACT queue
        # (transfers overlap LN2 + MLP1)
        pw2 = ctx.enter_context(tc.tile_pool(name="p_w2", bufs=4))
        b1c = load_cols(b1, 32, "b1c")
        B2 = load_bcast(b2, "B2")
        w2q = {}

        def load_w2(q):
            t = pw2.tile([128, 8, C], BF16, tag="w2q", name="w2q")
            nc.scalar.dma_start(out=t, in_=w2[q * 1024:(q + 1) * 1024, :]
                                .rearrange("(a p) c -> p a c", p=128))
            w2q[q] = t

        load_w2(0)
        load_w2(1)

        with tc.tile_pool(name="p_h2s", bufs=2) as ph2s:
            for j in range(NSLOT):
                stats = ph2s.tile([128, 2, 6], F32, tag="ln2_stats", name="ln2_stats")
                for s in range(2):
                    nc.vector.bn_stats(out=stats[:, s, :],
                                       in_=xmid[j][:, s * 512:(s + 1) * 512])
                mv = ph2s.tile([128, 2], F32, tag="ln2_mv", name="ln2_mv")
                nc.vector.bn_aggr(out=mv, in_=stats)
                std = ph2s.tile([128, 1], F32, tag="ln2_std", name="ln2_std")
                nc.scalar.activation(out=std, in_=mv[:, 1:2], func=AF.Sqrt, bias=eps_col)
                rstd = ph2s.tile([128, 1], F32, tag="ln2_rstd", name="ln2_rstd")
                nc.vector.reciprocal(out=rstd, in_=std)
                h2_rows = ph2s.tile([128, C], BF16, tag="h2_rows", name="h2_rows")
                nc.vector.tensor_scalar(out=h2_rows, in0=xmid[j], scalar1=mv[:, 0:1],
                                        scalar2=rstd, op0=OP.subtract, op1=OP.mult)
                nc.scalar.dma_start_transpose(out=h2T[:, :, j * 128:(j + 1) * 128],
                                              in_=h2_rows)

        # ==== phase 5: MLP1 + gelu -> mT ===================================
        with tc.tile_pool(name="ps_m1", bufs=4, space="PSUM") as ps_m1:
            for chunk in range(4):
                for co8 in range(8):
                    co = chunk * 8 + co8
                    ps = ps_m1.tile([128, 512], F32, tag="m1", name="m1")
                    for ci in range(8):
                        nc.tensor.matmul(
                            ps, lhsT=w1c[chunk][:, ci, co8 * 128:(co8 + 1) * 128],
                            rhs=h2T[:, ci, :], start=(ci == 0), stop=(ci == 7))
                    nc.scalar.activation(out=mT[:, co, :], in_=ps, func=AF.Gelu,
                                         bias=b1c[:, co:co + 1])
                if chunk + 2 < 4:
                    load_w1(chunk + 2)
                if chunk in (1, 2):
                    load_w2(chunk + 1)

        # ==== phase 6: MLP2 + residual -> out ==============================
        with tc.tile_pool(name="p_out", bufs=2) as pout, \
             tc.tile_pool(name="ps_m2", bufs=8, space="PSUM") as ps_m2:
            pss = [ps_m2.tile([128, 512], F32, tag="m2", name="m2")
                   for _ in range(8)]
            for q in range(4):
                for j in range(NSLOT):
                    for nt in range(2):
                        ps = pss[j * 2 + nt]
                        for ka in range(8):
                            ki = q * 8 + ka
                            nc.tensor.matmul(
                                ps, lhsT=mT[:, ki, j * 128:(j + 1) * 128],
                                rhs=w2q[q][:, ka, nt * 512:(nt + 1) * 512],
                                start=(ki == 0), stop=(ki == 31))
            for j in range(NSLOT):
                o_sb = pout.tile([128, C], F32, tag="o_sb", name="o_sb")
                for nt in range(2):
                    t1 = small.tile([128, 512], F32, tag="ot", name="ot")
                    nc.vector.tensor_add(t1, pss[j * 2 + nt],
                                         B2[:, nt * 512:(nt + 1) * 512])
                    nc.vector.tensor_add(
                        o_sb[:, nt * 512:(nt + 1) * 512], t1,
                        xmid[j][:, nt * 512:(nt + 1) * 512])
                nc.sync.dma_start(out=out[j * 128:(j + 1) * 128, :], in_=o_sb)

    _split_excess_waits(nc)
    return nc


def _split_excess_waits(nc, max_waits=1):
    """walrus rejects engine instructions with >1 sync wait. Hoist excess
    waits onto standalone EventSemaphore (pure-wait) instructions inserted
    just before the offending instruction on the same engine."""
    counter = 0
    for fn in nc.m.functions:
        for bb in fn.blocks:
            insts = bb.instructions
            i = 0
            while i < len(insts):
                inst = insts[i]
                si = getattr(inst, "sync_info", None)
                if os.environ.get("KEEP_DMA_WAITS") and \
                        type(inst).__name__ == "InstDMACopy":
                    i += 1
                    continue
                if (si is not None and si.on_wait
                        and len(si.on_wait) > max_waits):
                    waits = list(si.on_wait)
                    keep, extra = waits[-max_waits:], waits[:-max_waits]
                    for w in extra:
                        ev = mybir.InstEventSemaphore(
                            name=f"splitwait_{counter}", ins=[], outs=[])
                        counter += 1
                        ev.engine = inst.engine
                        ev.bass_nofuse = True
                        ev.sync_info = mybir.SyncInfo(on_wait=[w], on_update=[])
                        nc.register_instruction(ev)
                        insts.insert(i, ev)
                        i += 1
                    inst.sync_info = mybir.SyncInfo(
                        on_wait=keep, on_update=list(si.on_update))
                i += 1


_NC_CACHE = None


def _get_nc():
    global _NC_CACHE
    if _NC_CACHE is None:
        _NC_CACHE = build_nc()
    return _NC_CACHE


def _permute_wo_rows(wo) -> np.ndarray:
    """Reorder wo rows so slab index a=h8, partition p=hg*64+d maps to
    y channel (hg*8+h8)*64+d (the head-interleaved yT_all layout)."""
    wo = np.asarray(wo, np.float32)
    a = np.arange(C)
    p, blk = a % 128, a // 128          # row index within slab layout
    hg, d = p // 64, p % 64
    src_row = (hg * 8 + blk) * 64 + d
    out = np.empty_like(wo)
    out[a] = wo[src_row]
    return out


def make_masks(parity: int) -> np.ndarray:
    """[8,128,128] additive fp32 mask tiles for the MASKED (slot,kb) pairs."""
    tiles = np.zeros((8, 128, 128), np.float32)
    tri = np.where(np.arange(128)[:, None] <= np.arange(128)[None, :], 0.0, NEG)
    for i, (slot, kb) in enumerate(MASKED):
        g = QBLOCKS[parity][slot]
        if kb < g:
            tiles[i] = 0.0
        elif kb == g:
            tiles[i] = tri.astype(np.float32)
        else:
            tiles[i] = NEG
    return tiles


def make_in_maps(x: np.ndarray, weights: dict) -> list[dict]:
    bf = lambda a: np.ascontiguousarray(np.asarray(a, np.float32)).astype(
        ml_dtypes.bfloat16)
    f8 = lambda a: np.ascontiguousarray(
        np.asarray(a, np.float64) * WSCALE).astype(np.float32).astype(
        ml_dtypes.float8_e4m3)
    f32 = lambda a: np.ascontiguousarray(np.asarray(a, np.float32))
    g1 = np.asarray(weights["ln1_g"], np.float64)
    be1 = np.asarray(weights["ln1_b"], np.float64)
    g2 = np.asarray(weights["ln2_g"], np.float64)
    be2 = np.asarray(weights["ln2_b"], np.float64)
    # fold LN gamma into the next matmul's weights, LN beta into its bias
    def fold8(wname, bname):
        w = np.asarray(weights[wname], np.float64)
        b = np.asarray(weights[bname], np.float64)
        return f8(g1[:, None] * w), f32(b + be1 @ w)
    wq_f, bq_f = fold8("wq", "bq")
    wk_f, bk_f = fold8("wk", "bk")
    wv_f, bv_f = fold8("wv", "bv")
    w1_ = np.asarray(weights["w1"], np.float64)
    b1_ = np.asarray(weights["b1"], np.float64)
    w1_f, b1_f = bf(g2[:, None] * w1_), f32(b1_ + be2 @ w1_)
    shared = {
        "wq": wq_f, "bq": bq_f, "wk": wk_f, "bk": bk_f,
        "wv": wv_f, "bv": bv_f,
        "wo": f8(_permute_wo_rows(weights["wo"])), "bo": f32(weights["bo"]),
        "w1": w1_f, "b1": b1_f,
        "w2": bf(weights["w2"]), "b2": f32(weights["b2"]),
    }
    mask_by_parity = [make_masks(0), make_masks(1)]
    in_maps = []
    for core in range(8):
        b, parity = core // 2, core % 2
        qb = QBLOCKS[parity]
        xqg = np.concatenate([x[b, g * 128:(g + 1) * 128, :] for g in qb], axis=0)
        in_maps.append({
            "xb": f32(x[b]), "xq": f32(xqg), "masks": mask_by_parity[parity],
            **shared,
        })
    return in_maps


def assemble_out(results: list[dict]) -> np.ndarray:
    out = np.empty((B, T, C), np.float32)
    for core in range(8):
        b, parity = core // 2, core % 2
        o = np.asarray(results[core]["out"], np.float32)
        for j, g in enumerate(QBLOCKS[parity]):
            out[b, g * 128:(g + 1) * 128, :] = o[j * 128:(j + 1) * 128, :]
    return out


def kernel(**inputs) -> np.ndarray:
    x = np.asarray(inputs["x"], np.float32)
    nc = _get_nc()
    in_maps = make_in_maps(x, inputs)
    res = run_bass_kernel_spmd(nc, in_maps, list(range(8)))
    return assemble_out(res.results)


if __name__ == "__main__":
    _get_nc()
    print("built ok")


# revision 20
# speedup vs baseline: 1.1890x; 1.1890x over previous
"""Trainium2 Bass kernel for a dense transformer block (B=4, T=1024, C=1024, H=16).

Sharding: 2 cores per batch element (8 cores / 4 batches). Each core computes
K/V (+LN1) for its full batch but only 4 of the 8 query blocks of 128 rows.
Query blocks are interleaved ({7,4,3,0} on even cores, {6,5,2,1} on odd) so the
causal-attention work is balanced; the compiled program is identical on every
core (SPMD) - per-core behaviour comes only from input data (x slice, gathered
query rows, causal-mask tiles).

v3 layout/scheduling:
- LN gamma/beta folded into the next matmul's weights+bias on the HOST, so the
  device LN is only (x-m)*rstd.
- Q/K/V/O projections run in fp8e4m3 with DoubleRow (weights host-scaled x16,
  un-scaled in the bias-apply). Scores/AV and the MLP stay bf16.
- h transposes done by the DMA XBAR (dma_start_transpose) instead of the PE
  (XBAR is 16-bit, so transpose to a bf16 staging tile, then GpSimd casts to
  the fp8 hT).
- V projection is streamed per row-block right after that block's
  LN+transpose, K projection in two halves, so the tensor engine is densely
  busy early (keeps the HAM clock-gate at full rate).
- Two DMA queues: sync carries the x stream + small data, Activation carries
  all weight slabs, ordered so nothing compute-critical queues behind bulk
  weights. w1/w2 are prefetched during attention / MLP1.
- Attention scores for all 8 heads of a group land in one PSUM tile
  [128, 8, 128] (hh-major so the concurrent row-tiled pair writes distinct
  PSUM banks), exp'd by ONE 1024-wide activation per (slot, group, kb).
- Causal masks applied by a single broadcast scalar_tensor_tensor on the DVE.
- Softmax denominators: den row -> tiny PE transposes -> one batched [128,8]
  reciprocal -> PE transpose back -> K=1 matmul broadcast -> one fused
  normalize multiply per (slot, group) into the head-interleaved yT_all
  (wo rows host-permuted to match). Norm scratch shares the score pool ring.
"""
import os
import sys

for _p in ("/opt/trn_rl_repo", "/root/.axon_site/_ro/trn_rl_repo"):
    if os.path.isdir(_p) and _p not in sys.path:
        sys.path.insert(0, _p)

from contextlib import ExitStack

import ml_dtypes
import numpy as np

import concourse.bass as bass
import concourse.tile as tile
from concourse import mybir
from concourse.bass_utils import run_bass_kernel_spmd
from concourse.masks import make_identity

F32 = mybir.dt.float32
BF16 = mybir.dt.bfloat16
FP8 = mybir.dt.float8e4
PM_DR = mybir.MatmulPerfMode.DoubleRow
WSCALE = 16.0
WINV = 1.0 / WSCALE
AF = mybir.ActivationFunctionType
OP = mybir.AluOpType

B, T, C, H, D = 4, 1024, 1024, 16, 64
F = 4 * C                       # MLP hidden
NB = T // 128                   # 8 row blocks per batch
NSLOT = 4                       # query blocks per core
KMAX = [8, 6, 4, 2]             # k-blocks computed per slot (max over both cores)
QBLOCKS = [[7, 4, 3, 0], [6, 5, 2, 1]]  # global q-block per slot, by core parity
# (slot, kb) pairs that need a data mask (kb below min over parities: always allow)
MASKED = [(0, 6), (0, 7), (1, 4), (1, 5), (2, 2), (2, 3), (3, 0), (3, 1)]
EPS = 1e-5
NEG = -1e9


def build_nc():
    nc = bass.Bass("TRN2")

    # ---- DRAM I/O ----------------------------------------------------------
    xb = nc.dram_tensor("xb", [T, C], F32, kind="ExternalInput")     # full batch rows
    xq = nc.dram_tensor("xq", [512, C], F32, kind="ExternalInput")   # gathered q rows
    masks = nc.dram_tensor("masks", [8, 128, 128], F32, kind="ExternalInput")
    wq = nc.dram_tensor("wq", [C, C], FP8, kind="ExternalInput")
    wk = nc.dram_tensor("wk", [C, C], FP8, kind="ExternalInput")
    wv = nc.dram_tensor("wv", [C, C], FP8, kind="ExternalInput")
    wo = nc.dram_tensor("wo", [C, C], FP8, kind="ExternalInput")
    w1 = nc.dram_tensor("w1", [C, F], BF16, kind="ExternalInput")
    w2 = nc.dram_tensor("w2", [F, C], BF16, kind="ExternalInput")
    bq = nc.dram_tensor("bq", [C], F32, kind="ExternalInput")
    bk = nc.dram_tensor("bk", [C], F32, kind="ExternalInput")
    bv = nc.dram_tensor("bv", [C], F32, kind="ExternalInput")
    bo = nc.dram_tensor("bo", [C], F32, kind="ExternalInput")
    b1 = nc.dram_tensor("b1", [F], F32, kind="ExternalInput")
    b2 = nc.dram_tensor("b2", [C], F32, kind="ExternalInput")
    out = nc.dram_tensor("out", [512, C], F32, kind="ExternalOutput")

    with tile.TileContext(nc) as tc, ExitStack() as ctx:
        consts = ctx.enter_context(tc.tile_pool(name="consts", bufs=1))
        small = ctx.enter_context(tc.tile_pool(name="small", bufs=2))

        # ---- constants (no DMA, or tiny/early-needed loads) ---------------
        ones_row = consts.tile([1, 64], BF16, tag="ones_row", name="ones_row")
        nc.vector.memset(ones_row, 1.0)
        eps_col = consts.tile([128, 1], F32, tag="eps", name="eps")
        nc.vector.memset(eps_col, EPS)
        ident_f32 = consts.tile([128, 128], F32, tag="ident", name="ident")
        make_identity(nc, ident_f32)
        one_f32 = consts.tile([1, 1], F32, tag="one1", name="one1")
        nc.vector.memset(one_f32, 1.0)

        def load_cols(dram, nblk, tag):
            t = consts.tile([128, nblk], F32, tag=tag)
            nc.sync.dma_start(out=t, in_=dram.rearrange("(a p) -> p a", p=128))
            return t

        def load_bcast(dram, tag):
            t = consts.tile([128, C], F32, tag=tag)
            nc.sync.dma_start(
                out=t,
                in_=dram.rearrange("(one c) -> one c", one=1).partition_broadcast(128))
            return t

        xmid = [consts.tile([128, C], F32, tag=f"xmid{j}", name=f"xmid{j}") for j in range(4)]

        pw1 = ctx.enter_context(tc.tile_pool(name="p_w1", bufs=2))
        w1c = {}

        def load_w1(chunk):
            t = pw1.tile([128, 8, C], BF16, tag="w1c", name="w1c")
            nc.scalar.dma_start(out=t, in_=w1[:, chunk * C:(chunk + 1) * C]
                                .rearrange("(a p) c -> p a c", p=128))
            w1c[chunk] = t

        att_ctx = ExitStack()
        p_att = att_ctx.enter_context(tc.tile_pool(name="p_att", bufs=1))
        p_w = att_ctx.enter_context(tc.tile_pool(name="p_w", bufs=2))

        # attention operands (filled in phase 1)
        qT = p_att.tile([128, 8, 512], BF16, tag="qT", name="qT")
        kT = p_att.tile([128, 8, T], BF16, tag="kT", name="kT")
        vaug = [p_att.tile([128, 16, 65], BF16, tag=f"vaug{t}", name=f"vaug{t}") for t in range(8)]
        # yT_all[hg*64+d, h8, q] = normalized y for head hg*8+h8, dim d.
        # (wo rows are host-permuted to match this head-interleaved layout.)
        yT_all = p_att.tile([128, 8, 512], FP8, tag="yT", name="yT")

        def load_wslab(dram, name):
            t = p_w.tile([128, 8, C], FP8, tag="wslab", name=name)
            nc.scalar.dma_start(out=t, in_=dram.rearrange("(a p) c -> p a c", p=128))
            return t

        # first-needed loads: V weights (ACT queue) + V bias broadcast (sync)
        wv_sb = load_wslab(wv, "wv_sb")
        BV = load_bcast(bv, "BV")

        # ==== phase 1: streamed LN1 -> hT (DMA transpose) -> V/K/Q proj =====
        h1_ctx = ExitStack()
        p_h1 = h1_ctx.enter_context(tc.tile_pool(name="p_h1", bufs=1))
        hT = p_h1.tile([128, 8, 1536], FP8, tag="hT", name="hT")

        ph1s = h1_ctx.enter_context(tc.tile_pool(name="p_h1s", bufs=2))
        ps_qkv = h1_ctx.enter_context(tc.tile_pool(name="ps_qkv", bufs=4, space="PSUM"))

        def ln_block(x_ap, col_off):
            """LN (no gamma/beta) of [128, C] rows -> hT[:, :, col_off:+128]."""
            stats = ph1s.tile([128, 2, 6], F32, tag="ln_stats", name="ln_stats")
            for s in range(2):
                nc.vector.bn_stats(out=stats[:, s, :], in_=x_ap[:, s * 512:(s + 1) * 512])
            mv = ph1s.tile([128, 2], F32, tag="ln_mv", name="ln_mv")
            nc.vector.bn_aggr(out=mv, in_=stats)
            std = ph1s.tile([128, 1], F32, tag="ln_std", name="ln_std")
            nc.scalar.activation(out=std, in_=mv[:, 1:2], func=AF.Sqrt, bias=eps_col)
            rstd = ph1s.tile([128, 1], F32, tag="ln_rstd", name="ln_rstd")
            nc.vector.reciprocal(out=rstd, in_=std)
            h_rows = ph1s.tile([128, C], BF16, tag="h_rows", name="h_rows")
            nc.vector.tensor_scalar(out=h_rows, in0=x_ap, scalar1=mv[:, 0:1],
                                    scalar2=rstd, op0=OP.subtract, op1=OP.mult)
            stage_bf = ph1s.tile([128, 8, 128], BF16, tag="tstage", name="tstage")
            nc.sync.dma_start_transpose(out=stage_bf, in_=h_rows)
            nc.vector.tensor_copy(out=hT[:, :, col_off:col_off + 128], in_=stage_bf)

        def k_proj(nt):
            for co in range(8):
                ps = ps_qkv.tile([128, 512], F32, tag="mm", name="mm")
                for ci in range(0, 8, 2):
                    nc.tensor.matmul(
                        ps, lhsT=wk_sb[:, ci:ci + 2, co * 128:(co + 1) * 128],
                        rhs=hT[:, ci:ci + 2, nt * 512:(nt + 1) * 512],
                        start=(ci == 0), stop=(ci == 6), perf_mode=PM_DR)
                nc.vector.tensor_scalar(
                    out=kT[:, co, nt * 512:(nt + 1) * 512], in0=ps,
                    scalar1=WINV, scalar2=bkc[:, co:co + 1],
                    op0=OP.mult, op1=OP.add)

        for r in range(NB):
            x_t = ph1s.tile([128, C], F32, tag="x_t", name="x_t")
            nc.sync.dma_start(out=x_t, in_=xb[r * 128:(r + 1) * 128, :])
            ln_block(x_t, r * 128)
            # V projection for this row block (+bias), interleaved + ones col
            nc.vector.memset(vaug[r][:, :, 64:65], 1.0)
            for nt in range(2):
                ps = ps_qkv.tile([128, 512], F32, tag="mm", name="mm")
                for ci in range(0, 8, 2):
                    nc.tensor.matmul(
                        ps, lhsT=hT[:, ci:ci + 2, r * 128:(r + 1) * 128],
                        rhs=wv_sb[:, ci:ci + 2, nt * 512:(nt + 1) * 512],
                        start=(ci == 0), stop=(ci == 6), perf_mode=PM_DR)
                nc.vector.scalar_tensor_tensor(
                    out=vaug[r][:, nt * 8:(nt + 1) * 8, 0:64],
                    in0=ps.rearrange("p (h d) -> p h d", d=64),
                    scalar=WINV,
                    in1=BV[:, nt * 512:(nt + 1) * 512]
                        .rearrange("p (h d) -> p h d", d=64),
                    op0=OP.mult, op1=OP.add)
            # staggered loads on both queues, ordered by first use
            if r == 0:
                wk_sb = load_wslab(wk, "wk_sb")
                bkc = load_cols(bk, 8, "bkc")
            if r == 2:
                bqc = load_cols(bq, 8, "bqc")
            if r == 3:
                k_proj(0)
                wq_sb = load_wslab(wq, "wq_sb")
            if r == 4:
                xq_sb = p_att.tile([128, 4, C], F32, tag="xq", name="xq")
                nc.sync.dma_start(out=xq_sb,
                                  in_=xq.rearrange("(j p) c -> p j c", p=128))
            if r == 5:
                mask_sb = p_att.tile([128, 8, 128], F32, tag="masks", name="masks")
                nc.sync.dma_start(out=mask_sb, in_=masks.rearrange("i p q -> p i q"))
            if r == 6:
                BO = load_bcast(bo, "BO")
            if r == 7:
                k_proj(1)
                wo_sb = load_wslab(wo, "wo_sb")

        for j in range(NSLOT):
            ln_block(xq_sb[:, j, :], 1024 + j * 128)

        # Q^T -> [C, 512]
        for co in range(8):
            ps = ps_qkv.tile([128, 512], F32, tag="mm", name="mm")
            for ci in range(0, 8, 2):
                nc.tensor.matmul(ps, lhsT=wq_sb[:, ci:ci + 2, co * 128:(co + 1) * 128],
                                 rhs=hT[:, ci:ci + 2, 1024:1536],
                                 start=(ci == 0), stop=(ci == 6), perf_mode=PM_DR)
            nc.vector.tensor_scalar(out=qT[:, co, :], in0=ps,
                                    scalar1=WINV, scalar2=bqc[:, co:co + 1],
                                    op0=OP.mult, op1=OP.add)

        h1_ctx.close()

        # prefetch the first two w1 chunks during attention (ACT queue)
        load_w1(0)
        load_w1(1)

        # ==== phase 2: attention ===========================================
        mask_idx = {sk: i for i, sk in enumerate(MASKED)}
        with tc.tile_pool(name="p_exp", bufs=8) as pexp, \
             tc.tile_pool(name="p_sm", bufs=2) as psm, \
             tc.tile_pool(name="ps_s", bufs=2, space="PSUM") as ps_s, \
             tc.tile_pool(name="ps_y", bufs=2, space="PSUM") as ps_y:
            for j in range(NSLOT):
              for hg in range(2):             # head groups of 8
                # scores for all 8 heads of the group: [128, hh*4+hp4, 128]
                # (hh-major so the concurrent row-tiled pair writes distinct
                # PSUM banks)
                expS = [pexp.tile([128, 8, 128], BF16, tag="expS", name="expS")
                        for _ in range(KMAX[j])]
                for kb in range(KMAX[j]):
                    s_ps = ps_s.tile([128, 8, 128], F32, tag="s_ps", name="s_ps")
                    for hp4 in range(4):
                        hp = 4 * hg + hp4
                        for hh in range(2):
                            nc.tensor.matmul(
                                s_ps[:, hh * 4 + hp4, :],
                                lhsT=kT[hh * 64:(hh + 1) * 64, hp,
                                        kb * 128:(kb + 1) * 128],
                                rhs=qT[hh * 64:(hh + 1) * 64, hp,
                                       j * 128:(j + 1) * 128],
                                start=True, stop=True,
                                tile_position=(64 * hh, 0))
                    if (j, kb) in mask_idx:
                        mi = mask_idx[(j, kb)]
                        m_b = mask_sb[:, mi, :].unsqueeze(1) \
                            .broadcast_to([128, 8, 128])
                        sm = psm.tile([128, 8, 128], F32, tag="sm", name="sm")
                        nc.vector.scalar_tensor_tensor(
                            out=sm, in0=s_ps, scalar=0.125, in1=m_b,
                            op0=OP.mult, op1=OP.add)
                        nc.scalar.activation(out=expS[kb], in_=sm, func=AF.Exp)
                    else:
                        nc.scalar.activation(out=expS[kb], in_=s_ps,
                                             func=AF.Exp, scale=0.125)
                # AV for all 8 heads -> one PSUM tile [65, h8, 128]
                # (row 64 = softmax denominator from the vaug ones column)
                yaug8 = ps_y.tile([65, 8, 128], F32, tag="yaug8", name="yaug8")
                for h8 in range(8):
                    h = 8 * hg + h8
                    for kb in range(KMAX[j]):
                        nc.tensor.matmul(
                            yaug8[:, h8, :], lhsT=vaug[kb][:, h, :],
                            rhs=expS[kb][:, (h8 % 2) * 4 + h8 // 2, :],
                            start=(kb == 0), stop=(kb == KMAX[j] - 1))
                # normalization: den row -> transpose to q-partitions ->
                # batched reciprocal -> transpose back -> broadcast -> 1 mul.
                # scratch shares the score-pool ring (no extra PSUM banks).
                den_row = small.tile([1, 8, 128], F32, tag="den_row",
                                     name="den_row")
                nc.vector.tensor_copy(out=den_row, in_=yaug8[64:65, :, :])
                scr = ps_s.tile([128, 8, 128], F32, tag="s_ps", name="scr")
                dcol = scr[:, 0, 0:8]
                for h8 in range(8):
                    nc.tensor.transpose(dcol[:, h8:h8 + 1],
                                        den_row[0:1, h8, :], one_f32)
                rq = small.tile([128, 8], F32, tag="rq", name="rq")
                nc.vector.reciprocal(out=rq, in_=dcol)
                rT_ps = scr[0:8, 2, :]
                nc.tensor.transpose(rT_ps, rq, ident_f32)
                rT_sb = small.tile([8, 128], BF16, tag="rT_sb", name="rT_sb")
                nc.vector.tensor_copy(out=rT_sb, in_=rT_ps)
                rrow = small.tile([1, 8, 128], BF16, tag="rrow", name="rrow")
                nc.sync.dma_start(out=rrow[0:1, :, :], in_=rT_sb[:, :])
                for half in range(2):
                    nc.tensor.matmul(
                        scr[0:64, 4 * half:4 * half + 4, :],
                        lhsT=ones_row,
                        rhs=rrow[0:1, 4 * half:4 * half + 4, :],
                        start=True, stop=True)
                rb8_sb = small.tile([64, 8, 128], BF16, tag="rb8_sb",
                                    name="rb8_sb")
                nc.scalar.mul(rb8_sb, scr[0:64, :, :], 1.0)
                nc.vector.tensor_mul(
                    out=yT_all[hg * 64:(hg + 1) * 64, :,
                               j * 128:(j + 1) * 128],
                    in0=yaug8[0:64, :, :], in1=rb8_sb)

        # ==== phase 3: output projection + residual ========================
        with tc.tile_pool(name="ps_pr", bufs=4, space="PSUM") as ps_pr:
            for j in range(NSLOT):
                for nt in range(2):
                    ps = ps_pr.tile([128, 512], F32, tag="prj", name="prj")
                    for ci in range(0, 8, 2):
                        nc.tensor.matmul(
                            ps, lhsT=yT_all[:, ci:ci + 2, j * 128:(j + 1) * 128],
                            rhs=wo_sb[:, ci:ci + 2, nt * 512:(nt + 1) * 512],
                            start=(ci == 0), stop=(ci == 6), perf_mode=PM_DR)
                    t1 = small.tile([128, 512], F32, tag="prt", name="prt")
                    nc.vector.scalar_tensor_tensor(
                        out=t1, in0=ps, scalar=WINV,
                        in1=BO[:, nt * 512:(nt + 1) * 512],
                        op0=OP.mult, op1=OP.add)
                    nc.vector.tensor_add(
                        xmid[j][:, nt * 512:(nt + 1) * 512], t1,
                        xq_sb[:, j, nt * 512:(nt + 1) * 512])

        att_ctx.close()

        # ==== phase 4: LN2 (DMA transpose) + MLP ===========================
        p_mlp = ctx.enter_context(tc.tile_pool(name="p_mlp", bufs=1))
        h2T = p_mlp.tile([128, 8, 512], BF16, tag="h2T", name="h2T")
        mT = p_mlp.tile([128, 32, 512], BF16, tag="mT", name="mT")

        # w2 quarters: 4 fresh buffers dispatched up front on the ACT queue
        # (transfers overlap LN2 + MLP1)
        pw2 = ctx.enter_context(tc.tile_pool(name="p_w2", bufs=4))
        b1c = load_cols(b1, 32, "b1c")
        B2 = load_bcast(b2, "B2")
        w2q = {}

        def load_w2(q):
            t = pw2.tile([128, 8, C], BF16, tag="w2q", name="w2q")
            nc.scalar.dma_start(out=t, in_=w2[q * 1024:(q + 1) * 1024, :]
                                .rearrange("(a p) c -> p a c", p=128))
            w2q[q] = t

        load_w2(0)
        load_w2(1)

        with tc.tile_pool(name="p_h2s", bufs=2) as ph2s:
            for j in range(NSLOT):
                stats = ph2s.tile([128, 2, 6], F32, tag="ln2_stats", name="ln2_stats")
                for s in range(2):
                    nc.vector.bn_stats(out=stats[:, s, :],
                                       in_=xmid[j][:, s * 512:(s + 1) * 512])
                mv = ph2s.tile([128, 2], F32, tag="ln2_mv", name="ln2_mv")
                nc.vector.bn_aggr(out=mv, in_=stats)
                std = ph2s.tile([128, 1], F32, tag="ln2_std", name="ln2_std")
                nc.scalar.activation(out=std, in_=mv[:, 1:2], func=AF.Sqrt, bias=eps_col)
                rstd = ph2s.tile([128, 1], F32, tag="ln2_rstd", name="ln2_rstd")
                nc.vector.reciprocal(out=rstd, in_=std)
                h2_rows = ph2s.tile([128, C], BF16, tag="h2_rows", name="h2_rows")
                nc.vector.tensor_scalar(out=h2_rows, in0=xmid[j], scalar1=mv[:, 0:1],
                                        scalar2=rstd, op0=OP.subtract, op1=OP.mult)
                nc.sync.dma_start_transpose(out=h2T[:, :, j * 128:(j + 1) * 128],
                                              in_=h2_rows)

        # ==== phase 5: MLP1 + gelu -> mT ===================================
        with tc.tile_pool(name="ps_m1", bufs=4, space="PSUM") as ps_m1:
            for chunk in range(4):
                for co8 in range(8):
                    co = chunk * 8 + co8
                    ps = ps_m1.tile([128, 512], F32, tag="m1", name="m1")
                    for ci in range(8):
                        nc.tensor.matmul(
                            ps, lhsT=w1c[chunk][:, ci, co8 * 128:(co8 + 1) * 128],
                            rhs=h2T[:, ci, :], start=(ci == 0), stop=(ci == 7))
                    nc.scalar.activation(out=mT[:, co, :], in_=ps, func=AF.Gelu,
                                         bias=b1c[:, co:co + 1])
                if chunk + 2 < 4:
                    load_w1(chunk + 2)
                if chunk in (1, 2):
                    load_w2(chunk + 1)

        # ==== phase 6: MLP2 + residual -> out ==============================
        with tc.tile_pool(name="p_out", bufs=2) as pout, \
             tc.tile_pool(name="ps_m2", bufs=8, space="PSUM") as ps_m2:
            pss = [ps_m2.tile([128, 512], F32, tag="m2", name="m2")
                   for _ in range(8)]
            for q in range(4):
                for j in range(NSLOT):
                    for nt in range(2):
                        ps = pss[j * 2 + nt]
                        for ka in range(8):
                            ki = q * 8 + ka
                            nc.tensor.matmul(
                                ps, lhsT=mT[:, ki, j * 128:(j + 1) * 128],
                                rhs=w2q[q][:, ka, nt * 512:(nt + 1) * 512],
                                start=(ki == 0), stop=(ki == 31))
            for j in range(NSLOT):
                o_sb = pout.tile([128, C], F32, tag="o_sb", name="o_sb")
                for nt in range(2):
                    t1 = small.tile([128, 512], F32, tag="ot", name="ot")
                    nc.vector.tensor_add(t1, pss[j * 2 + nt],
                                         B2[:, nt * 512:(nt + 1) * 512])
                    nc.vector.tensor_add(
                        o_sb[:, nt * 512:(nt + 1) * 512], t1,
                        xmid[j][:, nt * 512:(nt + 1) * 512])
                nc.sync.dma_start(out=out[j * 128:(j + 1) * 128, :], in_=o_sb)

    _split_excess_waits(nc)
    return nc


def _split_excess_waits(nc, max_waits=1):
    """walrus rejects engine instructions with >1 sync wait. Hoist excess
    waits onto standalone EventSemaphore (pure-wait) instructions inserted
    just before the offending instruction on the same engine."""
    counter = 0
    for fn in nc.m.functions:
        for bb in fn.blocks:
            insts = bb.instructions
            i = 0
            while i < len(insts):
                inst = insts[i]
                si = getattr(inst, "sync_info", None)
                if os.environ.get("KEEP_DMA_WAITS") and \
                        type(inst).__name__ == "InstDMACopy":
                    i += 1
                    continue
                if (si is not None and si.on_wait
                        and len(si.on_wait) > max_waits):
                    waits = list(si.on_wait)
                    keep, extra = waits[-max_waits:], waits[:-max_waits]
                    for w in extra:
                        ev = mybir.InstEventSemaphore(
                            name=f"splitwait_{counter}", ins=[], outs=[])
                        counter += 1
                        ev.engine = inst.engine
                        ev.bass_nofuse = True
                        ev.sync_info = mybir.SyncInfo(on_wait=[w], on_update=[])
                        nc.register_instruction(ev)
                        insts.insert(i, ev)
                        i += 1
                    inst.sync_info = mybir.SyncInfo(
                        on_wait=keep, on_update=list(si.on_update))
                i += 1


_NC_CACHE = None


def _get_nc():
    global _NC_CACHE
    if _NC_CACHE is None:
        _NC_CACHE = build_nc()
    return _NC_CACHE


def _permute_wo_rows(wo) -> np.ndarray:
    """Reorder wo rows so slab index a=h8, partition p=hg*64+d maps to
    y channel (hg*8+h8)*64+d (the head-interleaved yT_all layout)."""
    wo = np.asarray(wo, np.float32)
    a = np.arange(C)
    p, blk = a % 128, a // 128          # row index within slab layout
    hg, d = p // 64, p % 64
    src_row = (hg * 8 + blk) * 64 + d
    out = np.empty_like(wo)
    out[a] = wo[src_row]
    return out


def make_masks(parity: int) -> np.ndarray:
    """[8,128,128] additive fp32 mask tiles for the MASKED (slot,kb) pairs."""
    tiles = np.zeros((8, 128, 128), np.float32)
    tri = np.where(np.arange(128)[:, None] <= np.arange(128)[None, :], 0.0, NEG)
    for i, (slot, kb) in enumerate(MASKED):
        g = QBLOCKS[parity][slot]
        if kb < g:
            tiles[i] = 0.0
        elif kb == g:
            tiles[i] = tri.astype(np.float32)
        else:
            tiles[i] = NEG
    return tiles


def make_in_maps(x: np.ndarray, weights: dict) -> list[dict]:
    bf = lambda a: np.ascontiguousarray(np.asarray(a, np.float32)).astype(
        ml_dtypes.bfloat16)
    f8 = lambda a: np.ascontiguousarray(
        np.asarray(a, np.float64) * WSCALE).astype(np.float32).astype(
        ml_dtypes.float8_e4m3)
    f32 = lambda a: np.ascontiguousarray(np.asarray(a, np.float32))
    g1 = np.asarray(weights["ln1_g"], np.float64)
    be1 = np.asarray(weights["ln1_b"], np.float64)
    g2 = np.asarray(weights["ln2_g"], np.float64)
    be2 = np.asarray(weights["ln2_b"], np.float64)
    # fold LN gamma into the next matmul's weights, LN beta into its bias
    def fold8(wname, bname):
        w = np.asarray(weights[wname], np.float64)
        b = np.asarray(weights[bname], np.float64)
        return f8(g1[:, None] * w), f32(b + be1 @ w)
    wq_f, bq_f = fold8("wq", "bq")
    wk_f, bk_f = fold8("wk", "bk")
    wv_f, bv_f = fold8("wv", "bv")
    w1_ = np.asarray(weights["w1"], np.float64)
    b1_ = np.asarray(weights["b1"], np.float64)
    w1_f, b1_f = bf(g2[:, None] * w1_), f32(b1_ + be2 @ w1_)
    shared = {
        "wq": wq_f, "bq": bq_f, "wk": wk_f, "bk": bk_f,
        "wv": wv_f, "bv": bv_f,
        "wo": f8(_permute_wo_rows(weights["wo"])), "bo": f32(weights["bo"]),
        "w1": w1_f, "b1": b1_f,
        "w2": bf(weights["w2"]), "b2": f32(weights["b2"]),
    }
    mask_by_parity = [make_masks(0), make_masks(1)]
    in_maps = []
    for core in range(8):
        b, parity = core // 2, core % 2
        qb = QBLOCKS[parity]
        xqg = np.concatenate([x[b, g * 128:(g + 1) * 128, :] for g in qb], axis=0)
        in_maps.append({
            "xb": f32(x[b]), "xq": f32(xqg), "masks": mask_by_parity[parity],
            **shared,
        })
    return in_maps


def assemble_out(results: list[dict]) -> np.ndarray:
    out = np.empty((B, T, C), np.float32)
    for core in range(8):
        b, parity = core // 2, core % 2
        o = np.asarray(results[core]["out"], np.float32)
        for j, g in enumerate(QBLOCKS[parity]):
            out[b, g * 128:(g + 1) * 128, :] = o[j * 128:(j + 1) * 128, :]
    return out


def kernel(**inputs) -> np.ndarray:
    x = np.asarray(inputs["x"], np.float32)
    nc = _get_nc()
    in_maps = make_in_maps(x, inputs)
    res = run_bass_kernel_spmd(nc, in_maps, list(range(8)))
    return assemble_out(res.results)


if __name__ == "__main__":
    _get_nc()
    print("built ok")
